# revision 4
# baseline (speedup 1.0000x reference)
"""CartBasisStressHead kernel for Trainium2 (8 NeuronCores, SPMD data-parallel).

Strategy
--------
Only 6 of the 9 m-rows of node_embedding are used: row 0 feeds a SiLU MLP
(per-node scalar), rows 4:9 feed a per-channel contraction (l=2 branch).
Nodes are sharded contiguously across 8 cores; segment sums are linear, so
the host adds per-shard partials.

The kernel is HBM-bound, so everything streamed is wired in fp8 (E4M3):
  * l=2 data uses a chain-of-4 compensated quantization (each node's rounding
    residual is folded into the next node of the same graph before rounding),
    cutting the segment-sum quantization error ~2x below plain fp8 rounding.
  * The segment sum itself runs on the PE in DoubleRow fp8 perf mode:
    256 nodes per pass (128 partitions x 2 pair lanes), with a 0/1 indicator
    matrix A[node, local_graph] as the stationary operand. DoubleRow only
    supports PSUM quadrant 0, so the two 320-column halves of the l=2
    features accumulate into two separate PSUM tiles.
  * The MLP runs x0 (fp8) against bf16 weights; per-node scalars come from a
    1-wide W3 matmul packed into spare PE column bands.

Inputs stream as 5-group superchunks, each split into a 1-group head DMA
(fast pipeline ramp) plus a 4-group rest DMA (~2.6 MB, near-peak HBM
efficiency); outputs stage in SBUF and store once per superchunk. Per-group
PE issue order interleaves the l=2 passes around the MLP layers so the
in-order PE queue never waits on the activation engine.

Host epilogue: scatter-add of per-group segment partials, contraction with
w_l2, bincount of per-node scalars, and the tiny (G,9)@(9,9) basis change.
"""

import sys

if "/opt/trn_rl_repo" not in sys.path:
    sys.path.insert(0, "/opt/trn_rl_repo")

import numpy as np
import ml_dtypes

import concourse.bacc as bacc
import concourse.tile as tile
from concourse import mybir
from concourse import bass_utils

_S2 = 2.0 ** -0.5
_S3 = 3.0 ** -0.5
_S6 = 6.0 ** -0.5
_CG = np.array([
    [_S3, 0, 0, 0, _S3, 0, 0, 0, _S3],
    [0, 0, 0, 0, 0, _S2, 0, -_S2, 0],
    [0, 0, -_S2, 0, 0, 0, _S2, 0, 0],
    [0, _S2, 0, -_S2, 0, 0, 0, 0, 0],
    [0, 0, _S2, 0, 0, 0, _S2, 0, 0],
    [0, 0, 0, 0, 0, _S2, 0, _S2, 0],
    [-_S6, 0, 0, 0, 2 * _S6, 0, 0, 0, -_S6],
    [0, _S2, 0, _S2, 0, 0, 0, 0, 0],
    [-_S2, 0, 0, 0, 0, 0, 0, 0, _S2],
], dtype=np.float32)  # (9, 9)

N_CORES = 8
P = 128          # SBUF partitions
NG = 1024        # nodes per group (one PSUM accumulation span)
DT = 4           # 256-node dtiles per group (DoubleRow pairs 2 nodes/lane)
ML2 = 640        # l=2 values per node (5 m-rows x 128 channels)
SCG = 5          # groups per superchunk (one input DMA / output store)
GPB = 5120       # el2 bytes per partition per group

F32 = mybir.dt.float32
BF16 = mybir.dt.bfloat16
F8 = mybir.dt.float8e4
DR = mybir.MatmulPerfMode.DoubleRow
WIRE8 = ml_dtypes.float8_e4m3

_BUILD_CACHE = {}


def _build(n_groups, W, n_real):
    key = (n_groups, W, n_real)
    if key in _BUILD_CACHE:
        return _BUILD_CACHE[key]

    n_sc = (n_groups + SCG - 1) // SCG
    T2 = n_groups * DT * 2

    nc = bacc.Bacc("TRN2", target_bir_lowering=False, debug=False,
                   num_devices=N_CORES)

    # inputs (host pre-packed; see kernel() for layouts)
    el2w = nc.dram_tensor("el2w", (n_sc, P, SCG * GPB), F8,
                          kind="ExternalInput").ap()
    x0w = nc.dram_tensor("x0w", (n_sc, P, SCG * NG), F8,
                         kind="ExternalInput").ap()
    lgid = nc.dram_tensor("lgid", (P, T2), F32, kind="ExternalInput").ap()
    iota_in = nc.dram_tensor("iota_in", (P, W), F32, kind="ExternalInput").ap()
    w1t = nc.dram_tensor("w1t", (P, P), BF16, kind="ExternalInput").ap()
    w2t = nc.dram_tensor("w2t", (P, P), BF16, kind="ExternalInput").ap()
    w3t = nc.dram_tensor("w3t", (P, 1), BF16, kind="ExternalInput").ap()
    b1 = nc.dram_tensor("b1c", (P, 1), F32, kind="ExternalInput").ap()
    b2 = nc.dram_tensor("b2c", (P, 1), F32, kind="ExternalInput").ap()
    # outputs
    scal = nc.dram_tensor("scal", (n_sc, SCG * NG), F32,
                          kind="ExternalOutput").ap()
    S_out = nc.dram_tensor("S_out", (n_sc, 64, SCG * 320), F32,
                           kind="ExternalOutput").ap()

    silu = mybir.ActivationFunctionType.Silu
    eq = mybir.AluOpType.is_equal

    with tile.TileContext(nc) as tc:
        with (
            tc.tile_pool(name="const", bufs=1) as cpool,
            tc.tile_pool(name="el2h", bufs=2) as el2hp,
            tc.tile_pool(name="el2r", bufs=2) as el2rp,
            tc.tile_pool(name="x0p", bufs=2) as x0p,
            tc.tile_pool(name="hp", bufs=6) as hp,
            tc.tile_pool(name="stp", bufs=2) as stp,
            tc.tile_pool(name="ph1", bufs=2, space="PSUM") as ph1p,
            tc.tile_pool(name="ph2", bufs=2, space="PSUM") as ph2p,
            tc.tile_pool(name="psc", bufs=1, space="PSUM") as pscp,
            tc.tile_pool(name="pSa", bufs=1, space="PSUM") as pSap,
            tc.tile_pool(name="pSb", bufs=1, space="PSUM") as pSbp,
        ):
            w1s = cpool.tile([P, P], BF16)
            w2s = cpool.tile([P, P], BF16)
            w3s = cpool.tile([P, 1], BF16)
            b1s = cpool.tile([P, 1], F32)
            b2s = cpool.tile([P, 1], F32)
            iotas = cpool.tile([P, W], F32)
            lgids = cpool.tile([P, T2], F32)
            nc.scalar.dma_start(out=w1s[:], in_=w1t)
            nc.scalar.dma_start(out=w2s[:], in_=w2t)
            nc.scalar.dma_start(out=w3s[:], in_=w3t)
            nc.scalar.dma_start(out=b1s[:], in_=b1)
            nc.scalar.dma_start(out=b2s[:], in_=b2)
            nc.scalar.dma_start(out=iotas[:], in_=iota_in)
            nc.scalar.dma_start(out=lgids[:], in_=lgid)

            # all indicator pair-matrices up front (one DVE instruction):
            # Aall[p, (g,d,i), w] = (iota[w] == lgid[p, (g,d,i)])
            Aall = cpool.tile([P, T2 * W], F8)
            nc.vector.tensor_tensor(
                out=Aall[:].rearrange("p (t w) -> p t w", t=T2, w=W),
                in0=iotas[:].unsqueeze(1).to_broadcast([P, T2, W]),
                in1=lgids[:].unsqueeze(2).to_broadcast([P, T2, W]),
                op=eq)

            for sc in range(n_sc):
                sc_g0 = sc * SCG
                sc_ng = min(SCG, n_groups - sc_g0)
                last_g = sc_g0 + sc_ng - 1
                lg_real = min(NG, n_real - last_g * NG)
                dt_last = (lg_real + 255) // 256
                sr_last = (lg_real + 511) // 512

                # head = group 0 of the superchunk (fast ramp), rest = tail
                el2h = el2hp.tile([P, GPB], F8, tag="el2h")
                nc.sync.dma_start(
                    out=el2h[:, :GPB if sc_ng > 1 else dt_last * 1280],
                    in_=el2w[sc][:, :GPB if sc_ng > 1 else dt_last * 1280])
                el2r = el2rp.tile([P, (SCG - 1) * GPB], F8, tag="el2r")
                if sc_ng > 1:
                    ext = (sc_ng - 2) * GPB + dt_last * 1280
                    nc.sync.dma_start(out=el2r[:, :ext],
                                      in_=el2w[sc][:, GPB: GPB + ext])
                x0c = x0p.tile([P, SCG * NG], F8, tag="x0c")
                xext = (sc_ng - 1) * NG + sr_last * 512
                nc.scalar.dma_start(out=x0c[:, :xext],
                                    in_=x0w[sc][:, :xext])

                scst = stp.tile([1, SCG * NG], F32, tag="scst")
                Sst = stp.tile([64, SCG * 320], F32, tag="Sst")

                for gl in range(sc_ng):
                    g = sc_g0 + gl
                    grp_real = min(NG, n_real - g * NG)
                    Sr = (grp_real + 511) // 512
                    Dr = (grp_real + 255) // 256
                    if gl == 0:
                        esrc = el2h
                        ebase = 0
                    else:
                        esrc = el2r
                        ebase = (gl - 1) * GPB

                    def l2mm(d):
                        t2i = (g * DT + d) * 2
                        Ad = Aall[:, t2i * W: (t2i + 2) * W] \
                            .rearrange("p (i w) -> p i w", i=2, w=W)
                        base = ebase + d * 1280
                        nc.tensor.matmul(
                            pSa[0:W, :], Ad,
                            esrc[:, base: base + 640]
                                .rearrange("p (i f) -> p i f", i=2, f=320),
                            start=(d == 0), stop=(d == Dr - 1),
                            perf_mode=DR, tile_position=(0, 0))
                        nc.tensor.matmul(
                            pSb[0:W, :], Ad,
                            esrc[:, base + 640: base + 1280]
                                .rearrange("p (i f) -> p i f", i=2, f=320),
                            start=(d == 0), stop=(d == Dr - 1),
                            perf_mode=DR, tile_position=(0, 0))

                    # ---- MLP layer 1 ----
                    h1list = []
                    for s in range(Sr):
                        nsl = slice(gl * NG + s * 512, gl * NG + (s + 1) * 512)
                        h1p = ph1p.tile([P, 512], F32, tag="h1p")
                        nc.tensor.matmul(h1p[:], w1s[:], x0c[:, nsl],
                                         start=True, stop=True)
                        h1s = hp.tile([P, 512], BF16, tag="h1s")
                        nc.scalar.activation(h1s[:], h1p[:], silu, bias=b1s[:])
                        h1list.append(h1s)

                    # ---- l=2 segment sum, first half of the dtiles ----
                    pSa = pSap.tile([32, 320], F32, tag="pSa")
                    pSb = pSbp.tile([32, 320], F32, tag="pSb")
                    for d in range((Dr + 1) // 2):
                        l2mm(d)

                    # ---- MLP layer 2 (h1 activations ready by now) ----
                    h2list = []
                    for s in range(Sr):
                        h2p = ph2p.tile([P, 512], F32, tag="h2p")
                        nc.tensor.matmul(h2p[:], w2s[:], h1list[s][:],
                                         start=True, stop=True)
                        h2s = hp.tile([P, 512], BF16, tag="h2s")
                        nc.scalar.activation(h2s[:], h2p[:], silu, bias=b2s[:])
                        h2list.append(h2s)

                    # ---- l=2 segment sum, second half ----
                    for d in range((Dr + 1) // 2, Dr):
                        l2mm(d)

                    # ---- per-node scalar: W3 @ h2, packed in col bands ----
                    scp = pscp.tile([P, 512], F32, tag="scp")
                    for s in range(Sr):
                        q = 64 + 32 * s
                        nc.tensor.matmul(scp[q:q + 1, :], w3s[:],
                                         h2list[s][:], start=True, stop=True,
                                         tile_position=(0, q))

                    # ---- stage results in SBUF ----
                    for s in range(Sr):
                        q = 64 + 32 * s
                        nc.vector.tensor_copy(
                            out=scst[:, gl * NG + s * 512:
                                     gl * NG + (s + 1) * 512],
                            in_=scp[q:q + 1, :])
                    nc.vector.tensor_copy(
                        out=Sst[0:32, gl * 320: (gl + 1) * 320], in_=pSa[:])
                    nc.vector.tensor_copy(
                        out=Sst[32:64, gl * 320: (gl + 1) * 320], in_=pSb[:])

                nc.scalar.dma_start(out=scal[sc: sc + 1, :], in_=scst[:])
                nc.scalar.dma_start(out=S_out[sc], in_=Sst[:])

    nc.compile()
    _BUILD_CACHE[key] = nc
    return nc


def _next_pow2(x):
    p = 8
    while p < x:
        p *= 2
    return p


def _host_reference(node_embedding, W1, b1, W2, b2, W3, b3, w_l2, batch,
                    natoms):
    """Pure-numpy fallback (only used for pathological graph layouts)."""
    G = natoms.shape[0]
    inv = 1.0 / natoms.astype(np.float32)
    x = node_embedding[:, 0, :]
    h = x @ W1.T + b1
    h = h / (1.0 + np.exp(-h))
    h = h @ W2.T + b2
    h = h / (1.0 + np.exp(-h))
    ns = (h @ W3.T + b3)[:, 0]
    ok = (batch >= 0) & (batch < G)
    bok = batch[ok]
    iso = np.bincount(bok, weights=ns[ok], minlength=G).astype(np.float32) \
        * inv
    nl2 = np.einsum("nmc,c->nm", node_embedding[:, 4:9, :], w_l2[0])
    aniso = np.stack(
        [np.bincount(bok, weights=nl2[ok, m], minlength=G)
         for m in range(5)], axis=1).astype(np.float32) * inv[:, None]
    dec = np.concatenate([iso[:, None], np.zeros((G, 3), np.float32), aniso],
                         axis=1)
    return (dec @ _CG).reshape(-1, 3, 3).astype(np.float32)


def _chain4_quant(el2, batch):
    """fp8 E4M3 quantization with 4-node error-feedback chains.

    Within each aligned run of 4 nodes, the rounding residual of node k is
    added to node k+1 before its rounding whenever both nodes belong to the
    same graph, so the graph-level segment sum sees ~1 rounding error per
    chain instead of 4."""
    n = el2.shape[0]
    v = el2.reshape(n // 4, 4, ML2)
    b4 = batch.reshape(n // 4, 4)
    out = np.empty((n // 4, 4, ML2), WIRE8)
    carry = np.zeros((n // 4, ML2), np.float32)
    for k in range(4):
        tgt = v[:, k] + carry
        q = tgt.astype(WIRE8)
        out[:, k] = q
        if k < 3:
            same = (b4[:, k] == b4[:, k + 1]).astype(np.float32)[:, None]
            carry = (tgt - q.astype(np.float32)) * same
    return out.reshape(n, ML2)


def kernel(node_embedding, W1, b1, W2, b2, W3, b3, w_l2, batch, natoms):
    node_embedding = np.asarray(node_embedding, dtype=np.float32)
    W1 = np.asarray(W1, dtype=np.float32)
    b1 = np.asarray(b1, dtype=np.float32)
    W2 = np.asarray(W2, dtype=np.float32)
    b2 = np.asarray(b2, dtype=np.float32)
    W3 = np.asarray(W3, dtype=np.float32)
    b3 = np.asarray(b3, dtype=np.float32)
    w_l2 = np.asarray(w_l2, dtype=np.float32)
    batch = np.asarray(batch).astype(np.int64)
    natoms_in = np.asarray(natoms)

    N = node_embedding.shape[0]
    G = natoms_in.shape[0]
    n_sh = (N + N_CORES - 1) // N_CORES
    n_sh = ((n_sh + 3) // 4) * 4       # chain alignment
    n_groups = (n_sh + NG - 1) // NG
    n_pad = n_groups * NG
    n_sc = (n_groups + SCG - 1) // SCG

    if N % 4 != 0:
        return _host_reference(node_embedding, W1, b1, W2, b2, W3, b3,
                               w_l2, batch, natoms_in)

    # per-core shard ranges and per-group base graph ids
    shards = []
    W_need = 8
    for c in range(N_CORES):
        n0 = min(c * n_sh, N)
        n1 = min(n0 + n_sh, N)
        b = batch[n0:n1]
        nreal = n1 - n0
        gbase = np.zeros(n_groups, np.int64)
        for grp in range(n_groups):
            lo = grp * NG
            hi = min(lo + NG, nreal)
            if lo < nreal:
                gbase[grp] = b[lo]
                span = int(b[hi - 1] - b[lo] + 1)
                W_need = max(W_need, span)
        shards.append((n0, n1, b, gbase))
    W = _next_pow2(W_need)
    if (W > 32 or not np.all(batch[:-1] <= batch[1:])
            or batch.min(initial=0) < 0 or batch.max(initial=0) >= G):
        return _host_reference(node_embedding, W1, b1, W2, b2, W3, b3,
                               w_l2, batch, natoms_in)

    nc = _build(n_groups, W, n_sh)

    WIRE16 = ml_dtypes.bfloat16
    w1t = np.ascontiguousarray(W1.T).astype(WIRE16)
    w2t = np.ascontiguousarray(W2.T).astype(WIRE16)
    w3t = np.ascontiguousarray(W3.T).astype(WIRE16)
    b1c = np.ascontiguousarray(b1[:, None])
    b2c = np.ascontiguousarray(b2[:, None])
    iota_c = np.ascontiguousarray(
        np.tile(np.arange(W, dtype=np.float32), (P, 1)))

    # global chain-compensated fp8 of the l=2 block + plain fp8 of x0
    el2q = _chain4_quant(
        node_embedding[:, 4:9, :].reshape(N, ML2), batch)
    x0q = node_embedding[:, 0, :].astype(WIRE8)

    in_maps = []
    for c in range(N_CORES):
        n0, n1, b, gbase = shards[c]
        nreal = n1 - n0
        n_pad_sc = n_sc * SCG * NG
        # x0 wire: [sc, c(128), node] channel-major
        x0T = np.zeros((P, n_pad_sc), WIRE8)
        x0T[:, :nreal] = x0q[n0:n1].T
        x0w = np.ascontiguousarray(
            x0T.reshape(P, n_sc, SCG * NG).transpose(1, 0, 2))
        # el2 wire: node j = g*1024 + d*256 + 2p + i lives at
        # [sc, p, (gl, d, h, i, f320)]
        el2 = np.zeros((n_pad_sc, ML2), WIRE8)
        el2[:nreal] = el2q[n0:n1]
        el2 = el2.reshape(n_sc, SCG, DT, P, 2, 2, 320)
        el2 = np.ascontiguousarray(el2.transpose(0, 3, 1, 2, 5, 4, 6)
                                   .reshape(n_sc, P, SCG * GPB))
        # local graph ids per (g, d, p, i)
        lg = np.full(n_pad, -1.0, np.float32)
        lg[:nreal] = (b - np.repeat(gbase, NG)[:nreal]).astype(np.float32)
        lg_t = np.ascontiguousarray(
            lg.reshape(n_groups, DT, P, 2).transpose(2, 0, 1, 3)
              .reshape(P, n_groups * DT * 2))
        in_maps.append({
            "el2w": el2, "x0w": x0w, "lgid": lg_t, "iota_in": iota_c,
            "w1t": w1t, "w2t": w2t, "w3t": w3t, "b1c": b1c, "b2c": b2c,
        })

    res = bass_utils.run_bass_kernel_spmd(nc, in_maps,
                                          core_ids=list(range(N_CORES)))

    # ---- host epilogue ----
    inv = (1.0 / natoms_in.astype(np.float32)).astype(np.float32)
    node_scalar = np.empty(N, np.float32)
    Sfull = np.zeros((G + 32, ML2), np.float32)
    for c in range(N_CORES):
        n0, n1, _, gbase = shards[c]
        nreal = n1 - n0
        sc = res.results[c]["scal"].reshape(-1)[:nreal]
        node_scalar[n0:n1] = sc
        Sc = res.results[c]["S_out"]        # (n_sc, 64, SCG*320) f32
        for grp in range(n_groups):
            if grp * NG < nreal:
                gb = int(gbase[grp])
                blk = Sc[grp // SCG][:, (grp % SCG) * 320:
                                     (grp % SCG + 1) * 320]
                Sfull[gb:gb + W, 0:320] += blk[0:W]
                Sfull[gb:gb + W, 320:640] += blk[32:32 + W]
    iso = np.bincount(batch, weights=node_scalar + b3[0], minlength=G)
    iso = iso.astype(np.float32) * inv
    aniso = (Sfull[:G].reshape(G, 5, P) @ w_l2[0]).astype(np.float32)
    aniso *= inv[:, None]
    dec = np.concatenate([iso[:, None], np.zeros((G, 3), np.float32), aniso],
                         axis=1)
    return (dec @ _CG).reshape(-1, 3, 3).astype(np.float32)
